# revision 1
# baseline (speedup 1.0000x reference)
"""Multi-head attention (B=4, S=2048, D=1024, H=16) on 8 Trainium2 cores.

Sharding: core c handles batch b = c//2 and head-group hg = c%2 (8 of the 16
heads, i.e. 512 of the 1024 projection dims).  Every core computes:

    Qc^T = (Wq_cols^T @ q[b]^T)           [512, 2048]   (proj-major layout)
    Kc^T = (Wk_cols^T @ k[b]^T)           [512, 2048]
    Vc   = (v[b] @ Wv_cols)               [2048, 512]
    S^T  = Kc_h @ Qc_h^T per head         (scores, transposed: [keys, queries])
    P^T  = exp(S^T/8 + maskbias)          (ACT engine, fused scale+mask)
    A^T  = V_h^T @ P^T   and  l = 1^T P^T (AV + denominator via matmul)
    A^T  = A^T * (1/l)                    (broadcast via selector matmul)
    out_partial = A_c @ Wo_rows           [2048, 1024]

Host sums the two partial outputs per batch (the "all-reduce after w_o")
and adds the folded bias bv @ Wo + bo.  Biases bq/bk are applied on-device
(per-partition adds); the mask is applied as an additive bias inside the
exp activation.

All matmuls run as float32r (fp32 storage, single-pass PE mode).
"""

import os
import numpy as np

B, S, D = 4, 2048, 1024
H, DK = 16, 64
P = 128
NCORES = 8
HPC = H // 2            # heads per core
PROJ = HPC * DK         # 512 projection dims per core
NDM = D // P            # 8 d_model chunks
NPC = PROJ // P         # 4 head-pair chunks
NSC = S // 512          # 4 seq chunks of 512
NSO = S // P            # 16 seq chunks of 128
NKC = S // P            # 16 key chunks of 128

MASK_NEG = -30000.0     # exp(x - 30000) == 0 in fp32 for any plausible x

_cache = {}


def _build():
    """Build + compile the per-core Bass program (same program on all cores)."""
    import concourse.bass as bass
    import concourse.bacc as bacc
    import concourse.mybir as mybir
    import concourse.tile as tile
    from contextlib import ExitStack

    f32 = mybir.dt.float32
    f32r = mybir.dt.float32r
    bf16 = mybir.dt.bfloat16
    AF = mybir.ActivationFunctionType
    MUL = mybir.AluOpType.mult

    nc = bacc.Bacc("TRN2", target_bir_lowering=False, debug=False,
                   num_devices=NCORES)

    qT = nc.dram_tensor("qT", [D, S], bf16, kind="ExternalInput").ap()
    kT = nc.dram_tensor("kT", [D, S], bf16, kind="ExternalInput").ap()
    vT = nc.dram_tensor("vT", [D, S], bf16, kind="ExternalInput").ap()
    wq = nc.dram_tensor("wq", [D, PROJ], bf16, kind="ExternalInput").ap()
    wk = nc.dram_tensor("wk", [D, PROJ], bf16, kind="ExternalInput").ap()
    wv = nc.dram_tensor("wv", [D, PROJ], bf16, kind="ExternalInput").ap()
    wo = nc.dram_tensor("wo", [PROJ, D], bf16, kind="ExternalInput").ap()
    bq2 = nc.dram_tensor("bq2", [P, NPC], f32, kind="ExternalInput").ap()
    bk2 = nc.dram_tensor("bk2", [P, NPC], f32, kind="ExternalInput").ap()
    mb = nc.dram_tensor("mb", [P, NKC], f32, kind="ExternalInput").ap()
    sel = nc.dram_tensor("sel", [P, 128], f32, kind="ExternalInput").ap()
    vones = nc.dram_tensor("vones", [P, NSO, HPC], bf16, kind="ExternalInput").ap()
    out = nc.dram_tensor("out", [S, D], f32, kind="ExternalOutput").ap()

    def r(x):
        return x

    with tile.TileContext(nc) as tc, ExitStack() as ctx:
        cpool = ctx.enter_context(tc.tile_pool(name="const", bufs=1))
        sel_sb = cpool.tile([P, 128], f32)
        nc.sync.dma_start(sel_sb[:], sel)
        mb_sb = cpool.tile([P, NKC], f32)
        nc.sync.dma_start(mb_sb[:], mb)
        bq_sb = cpool.tile([P, NPC], f32)
        nc.sync.dma_start(bq_sb[:], bq2)
        bk_sb = cpool.tile([P, NPC], f32)
        nc.sync.dma_start(bk_sb[:], bk2)

        wopool = ctx.enter_context(tc.tile_pool(name="wo", bufs=1))
        wo_sb = wopool.tile([P, NPC, D], bf16)
        nc.sync.dma_start(wo_sb[:], wo.rearrange("(o p) n -> p o n", p=P))

        # Long-lived SBUF pools allocated up front so later-phase tiles never
        # reuse phase-A addresses (address reuse adds false serialization).
        respool = ctx.enter_context(tc.tile_pool(name="res", bufs=1))
        QT_sb = respool.tile([P, NPC, S], bf16)   # [pair-chunk, seq]
        # K^T stored per head on the full 128-partition contraction range:
        # even heads carry data in rows 0-63 (rows 64-127 zero), odd heads
        # in rows 64-127 (rows 0-63 zero).  The S^T matmul is then a
        # standard-mode 128x128 matmul against the pair-stacked Q^T -- no
        # PE tiling modes anywhere (tiling-mode matmuls keep the PE clock
        # gate throttled at 1.2 GHz).
        KT_sb = respool.tile([P, HPC, S], bf16)
        nc.gpsimd.memset(KT_sb[:], 0.0)
        # V with an interleaved ones column per head: head h occupies
        # cols [h*65, h*65+64) and col h*65+64 == 1.0 (softmax denominator
        # rides along the AV matmul as output partition 64).
        V_sb = respool.tile([P, NSO, HPC * (DK + 1)], bf16)
        nc.sync.dma_start(
            V_sb.rearrange("p n (h w) -> p n h w", w=DK + 1)[:, :, :, DK], vones)

        atpool = ctx.enter_context(tc.tile_pool(name="at", bufs=1))
        AT_sb = atpool.tile([P, NPC, S], bf16)   # normalized A^T
        epool = ctx.enter_context(tc.tile_pool(name="expS", bufs=8))
        npool = ctx.enter_context(tc.tile_pool(name="norm", bufs=2))
        opool = ctx.enter_context(tc.tile_pool(name="ostage", bufs=4))

        # ---------------- Phase A: projections ----------------
        with ExitStack() as ctxA:
            wpool = ctxA.enter_context(tc.tile_pool(name="w", bufs=2))
            apool = ctxA.enter_context(tc.tile_pool(name="actT", bufs=2))
            psA = ctxA.enter_context(
                tc.tile_pool(name="psA", bufs=4, space="PSUM"))

            # Q^T and K^T: out[proj-chunk(128), seq(512)] = Wx^T @ xT
            for w_hbm, x_hbm, bias_sb, dst in (
                (wq, qT, bq_sb, QT_sb),
                (wk, kT, bk_sb, KT_sb),
            ):
                w_sb = wpool.tile([P, NDM, PROJ], bf16, tag="w", name="w_sb")
                nc.sync.dma_start(w_sb[:], w_hbm.rearrange("(o p) n -> p o n", p=P))
                for sc in range(NSC):
                    a_sb = apool.tile([P, NDM, 512], bf16, tag="a", name="a_sb")
                    nc.sync.dma_start(
                        a_sb[:],
                        x_hbm.rearrange("(o p) s -> p o s", p=P)[
                            :, :, sc * 512:(sc + 1) * 512],
                    )
                    for pc in range(NPC):
                        ps = psA.tile([P, 512], f32, tag="pp", name="psa")
                        for dc in range(NDM):
                            nc.tensor.matmul(
                                ps,
                                lhsT=r(w_sb[:, dc, pc * P:(pc + 1) * P]),
                                rhs=r(a_sb[:, dc, :]),
                                start=(dc == 0), stop=(dc == NDM - 1),
                            )
                        if dst is QT_sb:
                            nc.vector.tensor_scalar_add(
                                dst[:, pc, sc * 512:(sc + 1) * 512], ps,
                                bias_sb[:, pc:pc + 1])
                        else:
                            for half in range(2):
                                lo = half * 64
                                nc.vector.tensor_scalar_add(
                                    KT_sb[lo:lo + 64, 2 * pc + half,
                                          sc * 512:(sc + 1) * 512],
                                    ps[lo:lo + 64, :],
                                    bias_sb[lo:lo + 64, pc:pc + 1])

            # V: out[seq-chunk(128), proj(512)] = vT^T @ Wv
            w_sb = wpool.tile([P, NDM, PROJ], bf16, tag="w", name="w_sb")
            nc.sync.dma_start(w_sb[:], wv.rearrange("(o p) n -> p o n", p=P))
            for sc in range(NSC):
                a_sb = apool.tile([P, NDM, 512], bf16, tag="a", name="a_sb")
                nc.sync.dma_start(
                    a_sb[:],
                    vT.rearrange("(o p) s -> p o s", p=P)[
                        :, :, sc * 512:(sc + 1) * 512],
                )
                for so4 in range(4):
                    so = sc * 4 + so4
                    ps = psA.tile([P, 512], f32, tag="pp", name="psa")
                    for dc in range(NDM):
                        nc.tensor.matmul(
                            ps,
                            lhsT=r(a_sb[:, dc, so4 * P:(so4 + 1) * P]),
                            rhs=r(w_sb[:, dc, :]),
                            start=(dc == 0), stop=(dc == NDM - 1),
                        )
                    nc.vector.tensor_copy(
                        V_sb[:, so, :].rearrange(
                            "p (h w) -> p h w", w=DK + 1)[:, :, 0:DK],
                        ps.rearrange("p (h w) -> p h w", w=DK))

        # ---------------- Phase B: attention ----------------
        with ExitStack() as ctxB:
            with ExitStack() as ctxBi:
                psS = ctxBi.enter_context(
                    tc.tile_pool(name="psS", bufs=2, space="PSUM"))
                psAcc = ctxBi.enter_context(
                    tc.tile_pool(name="psAcc", bufs=1, space="PSUM"))

                for pr in range(NPC):           # head pairs
                    for qc in range(2):         # query 1024-chunks
                        avs = [
                            psAcc.tile([P, 1024], f32, tag=f"av{hi}",
                                       name=f"av{hi}")
                            for hi in range(2)
                        ]
                        for kc in range(NKC):   # key 128-chunks
                            es = []
                            for hi in range(2):
                                h = 2 * pr + hi
                                sp = psS.tile([P, 1024], f32, tag="s",
                                              name="sp")
                                for sub in range(2):
                                    nc.tensor.matmul(
                                        sp[:, sub * 512:(sub + 1) * 512],
                                        lhsT=r(KT_sb[:, h,
                                                     kc * P:(kc + 1) * P]),
                                        rhs=r(QT_sb[:, pr,
                                                    qc * 1024 + sub * 512:
                                                    qc * 1024 + (sub + 1) * 512]),
                                        start=True, stop=True,
                                    )
                                e = epool.tile([P, 1024], bf16, tag="e",
                                               name="e")
                                nc.scalar.activation(
                                    e, sp, AF.Exp,
                                    bias=mb_sb[:, kc:kc + 1],
                                    scale=float(1.0 / np.sqrt(DK)),
                                )
                                es.append(e)
                            for hi in range(2):
                                h = 2 * pr + hi
                                for sub in range(2):
                                    nc.tensor.matmul(
                                        avs[hi][0:DK + 1,
                                                sub * 512:(sub + 1) * 512],
                                        lhsT=r(V_sb[:, kc,
                                                    h * (DK + 1):
                                                    (h + 1) * (DK + 1)]),
                                        rhs=r(es[hi][:, sub * 512:
                                                     (sub + 1) * 512]),
                                        start=(kc == 0),
                                        stop=(kc == NKC - 1),
                                    )
                        # normalization: A^T *= 1/l (broadcast via selector mm)
                        Lsb4 = npool.tile([P, 1024], f32, tag="lsb",
                                          name="Lsb4")
                        nc.gpsimd.memset(Lsb4[:], 0.0)
                        for hi in range(2):
                            nc.vector.tensor_copy(
                                Lsb4[hi * 32:hi * 32 + 1, :],
                                avs[hi][DK:DK + 1, :])
                        bc = psS.tile([P, 1024], f32, tag="s", name="bc")
                        for sub in range(2):
                            nc.tensor.matmul(
                                bc[:, sub * 512:(sub + 1) * 512],
                                lhsT=r(sel_sb[:]),
                                rhs=r(Lsb4[:, sub * 512:(sub + 1) * 512]),
                                start=True, stop=True,
                            )
                        rc = npool.tile([P, 1024], f32, tag="rc", name="rc")
                        nc.vector.reciprocal(rc, bc)
                        for hi in range(2):
                            nc.vector.tensor_tensor(
                                AT_sb[hi * 64:(hi + 1) * 64, pr,
                                      qc * 1024:(qc + 1) * 1024],
                                avs[hi][0:64, :],
                                rc[hi * 64:(hi + 1) * 64, :], MUL)

            # ---------------- Phase C: output projection ----------------
            with ExitStack() as ctxC:
                psC = ctxC.enter_context(
                    tc.tile_pool(name="psC", bufs=4, space="PSUM"))
                for so in range(NSO):
                    for oc in range(2):
                        ps = psC.tile([P, 512], f32, tag="po", name="pso")
                        for pc in range(NPC):
                            nc.tensor.matmul(
                                ps,
                                lhsT=r(AT_sb[:, pc, so * P:(so + 1) * P]),
                                rhs=r(wo_sb[:, pc, oc * 512:(oc + 1) * 512]),
                                start=(pc == 0), stop=(pc == NPC - 1),
                            )
                        ost = opool.tile([P, 512], f32, tag="o", name="ost")
                        nc.vector.tensor_copy(ost, ps)
                        nc.sync.dma_start(
                            out[so * P:(so + 1) * P, oc * 512:(oc + 1) * 512],
                            ost)

    nc.compile()
    return nc


def _get_nc():
    if "nc" not in _cache:
        _cache["nc"] = _build()
    return _cache["nc"]


def make_in_maps(q, k, v, mask, Wq, bq, Wk, bk, Wv, bv, Wo, bo):
    """Host-side sharding: slice/transpose the full inputs per core."""
    import ml_dtypes
    f = np.float32
    bf = ml_dtypes.bfloat16
    q = np.asarray(q, dtype=f)
    k = np.asarray(k, dtype=f)
    v = np.asarray(v, dtype=f)
    Wq = np.asarray(Wq, dtype=f)
    Wk = np.asarray(Wk, dtype=f)
    Wv = np.asarray(Wv, dtype=f)
    Wo = np.asarray(Wo, dtype=f)
    bq = np.asarray(bq, dtype=f)
    bk = np.asarray(bk, dtype=f)
    mask = np.asarray(mask)

    sel = np.zeros((P, 128), dtype=f)
    sel[0, 0:64] = 1.0
    sel[32, 64:128] = 1.0

    in_maps = []
    for c in range(NCORES):
        b, hg = divmod(c, 2)
        cols = slice(hg * PROJ, (hg + 1) * PROJ)
        mbias = np.where(mask[b, 0, 0, :] == 0, f(MASK_NEG), f(0.0)).astype(f)
        in_maps.append({
            "qT": np.ascontiguousarray(q[b].T).astype(bf),
            "kT": np.ascontiguousarray(k[b].T).astype(bf),
            "vT": np.ascontiguousarray(v[b].T).astype(bf),
            "wq": np.ascontiguousarray(Wq[:, cols]).astype(bf),
            "wk": np.ascontiguousarray(Wk[:, cols]).astype(bf),
            "wv": np.ascontiguousarray(Wv[:, cols]).astype(bf),
            "wo": np.ascontiguousarray(Wo[cols, :]).astype(bf),
            "bq2": np.ascontiguousarray(bq[cols].reshape(NPC, P).T),
            "bk2": np.ascontiguousarray(bk[cols].reshape(NPC, P).T),
            "mb": np.ascontiguousarray(mbias.reshape(NKC, P).T),
            "sel": sel,
            "vones": np.ones((P, NSO, HPC), dtype=bf),
        })
    return in_maps


def combine_outputs(parts, Wv_bv_Wo_bo):
    """Host-side unshard: sum the two head-group partials per batch, add the
    folded bias bv @ Wo + bo."""
    bv, Wo, bo = Wv_bv_Wo_bo
    bo_eff = (np.asarray(bv, np.float32) @ np.asarray(Wo, np.float32)
              + np.asarray(bo, np.float32))
    out = np.empty((B, S, D), dtype=np.float32)
    for b in range(B):
        out[b] = parts[2 * b] + parts[2 * b + 1] + bo_eff
    return out


def _install_axon_ntff_hook():
    """The agent image's antenv lacks axon_hooks; synthesize it and register
    the ctypes NTFF profile hook from trn_boot so trace=True works."""
    import sys
    import types
    if "antenv.axon_hooks" in sys.modules:
        return
    try:
        from trn_agent_boot.trn_boot import _ntff_profile_via_ctypes
        hook = _ntff_profile_via_ctypes("/opt/axon/libaxon_pjrt.so")
    except Exception:
        hook = None
    mod = types.ModuleType("antenv.axon_hooks")
    mod._hook = hook
    mod.get_axon_ntff_profile_hook = lambda: mod._hook
    mod.set_axon_ntff_profile_hook = lambda h: setattr(mod, "_hook", h)
    sys.modules["antenv.axon_hooks"] = mod
    # upload_artifacts wants a fish bucket; keep artifacts local instead.
    import concourse.bass_utils as bu
    bu.upload_artifacts = lambda tmpdir: str(tmpdir)


def kernel(q, k, v, mask, Wq, bq, Wk, bk, Wv, bv, Wo, bo):
    from concourse.bass_utils import run_bass_kernel_spmd

    nc = _get_nc()
    in_maps = make_in_maps(q, k, v, mask, Wq, bq, Wk, bk, Wv, bv, Wo, bo)
    trace = bool(int(os.environ.get("KERNEL_TRACE", "0")))
    if trace:
        try:
            _install_axon_ntff_hook()
        except Exception:
            trace = False
    try:
        res = run_bass_kernel_spmd(
            nc, in_maps, list(range(NCORES)), trace=trace,
            tmpdir=os.environ.get("KERNEL_TRACE_DIR") or None)
    except Exception:
        if not trace:
            raise
        # Trace machinery failed; rerun without it so results still flow.
        res = run_bass_kernel_spmd(nc, in_maps, list(range(NCORES)), trace=False)
    _cache["last_result"] = res
    parts = [res.results[c]["out"] for c in range(NCORES)]
    return combine_outputs(parts, (bv, Wo, bo))



# revision 2
# speedup vs baseline: 1.0045x; 1.0045x over previous
"""Multi-head attention (B=4, S=2048, D=1024, H=16) on 8 Trainium2 cores, v3.

Core c: batch c//2, head-group c%2 (8 heads, 512 proj dims).  v3 fuses all
phases into one dense PE/ACT stream built from single-head attention
iterations (pr, qc, hi):

  - scores: lhsT = packed K^T (both heads' dims, K=128 standard matmul),
    rhs = per-head zero-padded Q^T scratch (QTp; the two scratch buffers
    alternate by head parity so their zero halves persist).  No PE tiling
    modes anywhere (they hold the PE clock gate at 1.2 GHz).
  - AV + softmax denominator via the interleaved ones-column (M=65).
  - PSUM: psS 2x[128,1024] (scores/exp, double-buffered) + psAcc [65,1024]
    (AV accumulator) + psA 2x[128,512] dedicated to filler matmuls = 8 banks.
  - projections (K/Q for next pair, V JIT in pair 0, first half of the
    output projection in pair 3) run as atomic filler chunks in dedicated
    PSUM, so the exp stream's double-buffering is never disturbed.
  - copy-first normalization: l row + unnormalized A^T leave PSUM right
    after the last AV; selector-broadcast + reciprocal + scale run deferred
    inside the next iteration.  Output is written bf16 (host upcasts).
"""

import os
import numpy as np

B, S, D = 4, 2048, 1024
H, DK = 16, 64
P = 128
NCORES = 8
HPC = H // 2
PROJ = HPC * DK
NDM = D // P
NPC = PROJ // P
NSC = S // 512
NSO = S // P
NKC = S // P
VW = DK + 1             # 65 cols per head in V_sb: 64 data + ones

MASK_NEG = -30000.0

_cache = {}


def _build():
    import concourse.bacc as bacc
    import concourse.mybir as mybir
    import concourse.tile as tile
    from contextlib import ExitStack

    f32 = mybir.dt.float32
    bf16 = mybir.dt.bfloat16
    AF = mybir.ActivationFunctionType
    MUL = mybir.AluOpType.mult

    nc = bacc.Bacc("TRN2", target_bir_lowering=False, debug=False,
                   num_devices=NCORES)

    qT = nc.dram_tensor("qT", [D, S], bf16, kind="ExternalInput").ap()
    kT = nc.dram_tensor("kT", [D, S], bf16, kind="ExternalInput").ap()
    vT = nc.dram_tensor("vT", [D, S], bf16, kind="ExternalInput").ap()
    wqr = nc.dram_tensor("wqr", [NPC, P, NDM, P], bf16, kind="ExternalInput").ap()
    wkr = nc.dram_tensor("wkr", [NPC, P, NDM, P], bf16, kind="ExternalInput").ap()
    wv = nc.dram_tensor("wv", [D, PROJ], bf16, kind="ExternalInput").ap()
    wo = nc.dram_tensor("wo", [PROJ, D], bf16, kind="ExternalInput").ap()
    bq2 = nc.dram_tensor("bq2", [P, NPC], f32, kind="ExternalInput").ap()
    bk2 = nc.dram_tensor("bk2", [P, NPC], f32, kind="ExternalInput").ap()
    mb = nc.dram_tensor("mb", [P, NKC], f32, kind="ExternalInput").ap()
    sel = nc.dram_tensor("sel", [P, 128], bf16, kind="ExternalInput").ap()
    vones = nc.dram_tensor("vones", [P, NSO, HPC], bf16,
                           kind="ExternalInput").ap()
    out = nc.dram_tensor("out", [S, D], bf16, kind="ExternalOutput").ap()

    with tile.TileContext(nc) as tc, ExitStack() as ctx:
        cpool = ctx.enter_context(tc.tile_pool(name="const", bufs=1))
        sel_sb = cpool.tile([P, 128], bf16)
        nc.sync.dma_start(sel_sb[:], sel)
        mb_sb = cpool.tile([P, NKC], f32)
        nc.sync.dma_start(mb_sb[:], mb)
        bq_sb = cpool.tile([P, NPC], f32)
        nc.sync.dma_start(bq_sb[:], bq2)
        bk_sb = cpool.tile([P, NPC], f32)
        nc.sync.dma_start(bk_sb[:], bk2)
        Lsb = cpool.tile([P, 1024], bf16)
        nc.gpsimd.memset(Lsb[:], 0.0)
        QTp = [cpool.tile([P, 1024], bf16, name=f"qtp{i}") for i in range(2)]
        nc.gpsimd.memset(QTp[0][:], 0.0)
        nc.gpsimd.memset(QTp[1][:], 0.0)

        rpool = ctx.enter_context(tc.tile_pool(name="res", bufs=1))
        kT_sb = rpool.tile([P, NDM, S], bf16)
        nc.sync.dma_start(kT_sb[:], kT.rearrange("(o p) s -> p o s", p=P))
        qT_sb = rpool.tile([P, NDM, S], bf16)
        nc.sync.dma_start(qT_sb[:], qT.rearrange("(o p) s -> p o s", p=P))
        QT_sb = rpool.tile([P, NPC, S], bf16)
        KT_sb = rpool.tile([P, NPC, S], bf16)
        AT_sb = rpool.tile([P, NPC, S], bf16)
        V_sb = rpool.tile([P, NSO, HPC * VW], bf16)
        nc.sync.dma_start(
            V_sb.rearrange("p n (h w) -> p n h w", w=VW)[:, :, :, DK], vones)

        wpool = ctx.enter_context(tc.tile_pool(name="w", bufs=2))
        stpool = ctx.enter_context(tc.tile_pool(name="st", bufs=2))
        epool = ctx.enter_context(tc.tile_pool(name="expS", bufs=2))
        rcpool = ctx.enter_context(tc.tile_pool(name="rc", bufs=1))
        bspool = ctx.enter_context(tc.tile_pool(name="bs", bufs=1))
        opool = ctx.enter_context(tc.tile_pool(name="ostage", bufs=2))
        psS = ctx.enter_context(tc.tile_pool(name="psS", bufs=2, space="PSUM"))
        psA = ctx.enter_context(tc.tile_pool(name="psA", bufs=2, space="PSUM"))
        psAcc = ctx.enter_context(tc.tile_pool(name="psAcc", bufs=1,
                                               space="PSUM"))

        def kq_fills(pr):
            wk_sb = wpool.tile([P, NDM, P], bf16, tag="w", name="wk_sb")
            nc.sync.dma_start(wk_sb[:], wkr[pr])
            wq_sb = wpool.tile([P, NDM, P], bf16, tag="w", name="wq_sb")
            nc.sync.dma_start(wq_sb[:], wqr[pr])

            def chunk(is_k, sc):
                w_sb = wk_sb if is_k else wq_sb
                src = kT_sb if is_k else qT_sb
                ps = psA.tile([P, 512], f32, tag="a", name="psp")
                for dc in range(NDM):
                    nc.tensor.matmul(
                        ps,
                        lhsT=w_sb[:, dc, :],
                        rhs=src[:, dc, sc * 512:(sc + 1) * 512],
                        start=(dc == 0), stop=(dc == NDM - 1),
                    )
                dst = KT_sb if is_k else QT_sb
                bias = bk_sb if is_k else bq_sb
                nc.vector.tensor_scalar_add(
                    dst[:, pr, sc * 512:(sc + 1) * 512], ps,
                    bias[:, pr:pr + 1])

            return [(lambda is_k=is_k, sc=sc: chunk(is_k, sc))
                    for is_k in (True, False) for sc in range(NSC)]

        def v_chunk(vT_sb, wv_sb, so):
            ps = psA.tile([P, 512], f32, tag="a", name="psv")
            for dc in range(NDM):
                nc.tensor.matmul(
                    ps,
                    lhsT=vT_sb[:, dc, so * P:(so + 1) * P],
                    rhs=wv_sb[:, dc, :],
                    start=(dc == 0), stop=(dc == NDM - 1),
                )
            nc.vector.tensor_copy(
                V_sb[:, so, :].rearrange("p (h w) -> p h w", w=VW)[:, :, 0:DK],
                ps.rearrange("p (h w) -> p h w", w=DK))

        def c_fill(wo_sb, so, oc):
            ps = psA.tile([P, 512], f32, tag="a", name="psc")
            for pc in range(NPC):
                nc.tensor.matmul(
                    ps,
                    lhsT=AT_sb[:, pc, so * P:(so + 1) * P],
                    rhs=wo_sb[:, pc, oc * 512:(oc + 1) * 512],
                    start=(pc == 0), stop=(pc == NPC - 1),
                )
            ost = opool.tile([P, 512], bf16, tag="o", name="ost")
            nc.vector.tensor_copy(ost, ps)
            nc.sync.dma_start(
                out[so * P:(so + 1) * P, oc * 512:(oc + 1) * 512], ost)

        pending = []

        def norm_part2():
            if not pending:
                return
            pr, qc = pending.pop()
            bc = psS.tile([P, 1024], f32, tag="s", name="bc")
            for sub in range(2):
                nc.tensor.matmul(
                    bc[:, sub * 512:(sub + 1) * 512],
                    lhsT=sel_sb[:],
                    rhs=Lsb[:, sub * 512:(sub + 1) * 512],
                    start=True, stop=True,
                )
            bs = bspool.tile([P, 1024], f32, tag="bs", name="bs")
            nc.vector.tensor_copy(bs, bc)
            rc = rcpool.tile([P, 1024], bf16, tag="rc", name="rc")
            with nc.allow_low_precision(reason="1/l in bf16 is within rtol"):
                nc.vector.reciprocal(rc, bs)
            for hi in range(2):
                lo = hi * 64
                dst = AT_sb[lo:lo + 64, pr, qc * 1024:(qc + 1) * 1024]
                nc.vector.tensor_tensor(dst, dst, rc[lo:lo + 64, :], MUL)

        def qtp_prep(pr, qc, hi):
            """Stage the zero-padded per-head Q^T slice for (pr, qc, hi)."""
            qoff = qc * 1024
            nc.vector.tensor_copy(QTp[hi][hi * 64:hi * 64 + 64, :],
                                  QT_sb[hi * 64:hi * 64 + 64, pr,
                                        qoff:qoff + 1024])

        def attn_iter(pr, qc, hi, pre=None, fills=(), fill_at=4, prep=None):
            fills = list(fills)
            h = 2 * pr + hi
            qoff = qc * 1024
            qtp = QTp[hi]
            avs = psAcc.tile([P, 1024], f32, tag="av", name="avs")
            for kc in range(NKC):
                if pre is not None:
                    pre(kc)
                sp = psS.tile([P, 1024], f32, tag="s", name="sp")
                for sub in range(2):
                    nc.tensor.matmul(
                        sp[:, sub * 512:(sub + 1) * 512],
                        lhsT=KT_sb[:, pr, kc * P:(kc + 1) * P],
                        rhs=qtp[:, sub * 512:(sub + 1) * 512],
                        start=True, stop=True,
                    )
                e = epool.tile([P, 1024], bf16, tag="e", name="e")
                nc.scalar.activation(
                    e, sp, AF.Exp,
                    bias=mb_sb[:, kc:kc + 1],
                    scale=float(1.0 / np.sqrt(DK)),
                )
                for sub in range(2):
                    nc.tensor.matmul(
                        avs[0:DK + 1, sub * 512:(sub + 1) * 512],
                        lhsT=V_sb[:, kc, h * VW:(h + 1) * VW],
                        rhs=e[:, sub * 512:(sub + 1) * 512],
                        start=(kc == 0), stop=(kc == NKC - 1),
                    )
                if kc == 3:
                    norm_part2()
                if kc == 8 and prep is not None:
                    prep()
                if kc >= fill_at and fills:
                    fills.pop(0)()
            while fills:
                fills.pop(0)()
            # evacuate PSUM in one staged copy; fan out off the critical path
            st = stpool.tile([P, 1024], bf16, tag="st", name="st")
            nc.vector.tensor_copy(st[0:DK + 1, :], avs[0:DK + 1, :])
            nc.vector.tensor_copy(Lsb[32 * hi:32 * hi + 1, :],
                                  st[DK:DK + 1, :])
            nc.vector.tensor_copy(
                AT_sb[64 * hi:64 * hi + 64, pr, qoff:qoff + 1024],
                st[0:DK, :])
            if hi == 1:
                pending.append((pr, qc))

        # ---------------- schedule ----------------
        fills0 = kq_fills(0)
        with tc.tile_pool(name="vres", bufs=1) as vpool:
            vT_sb = vpool.tile([P, NDM, S], bf16)
            nc.sync.dma_start(vT_sb[:], vT.rearrange("(o p) s -> p o s", p=P))
            wv_sb = vpool.tile([P, NDM, PROJ], bf16)
            nc.sync.dma_start(wv_sb[:], wv.rearrange("(o p) n -> p o n", p=P))

            for f in fills0:
                f()
            v_chunk(vT_sb, wv_sb, 0)

            def pre_v(kc):
                if kc + 1 < NSO:
                    v_chunk(vT_sb, wv_sb, kc + 1)

            fills1 = kq_fills(1)
            seq = [(0, 0, 0), (0, 0, 1), (0, 1, 0), (0, 1, 1)]
            preps = [
                (lambda i=i: qtp_prep(*seq[i + 1])) for i in range(3)
            ] + [lambda: qtp_prep(1, 0, 0)]
            qtp_prep(0, 0, 0)
            attn_iter(0, 0, 0, pre=pre_v, prep=preps[0])
            attn_iter(0, 0, 1, fills=fills1[:3], fill_at=5, prep=preps[1])
            attn_iter(0, 1, 0, fills=fills1[3:6], fill_at=5, prep=preps[2])
            attn_iter(0, 1, 1, fills=fills1[6:], fill_at=5, prep=preps[3])

        with tc.tile_pool(name="wores", bufs=1) as wopool:
            wo_sb = wopool.tile([P, NPC, D], bf16)
            nc.sync.dma_start(wo_sb[:], wo.rearrange("(o p) n -> p o n", p=P))

            seq2 = [(pr, qc, hi) for pr in (1, 2, 3)
                    for qc in (0, 1) for hi in (0, 1)]
            cfills = [(lambda so=so, oc=oc: c_fill(wo_sb, so, oc))
                      for so in range(NSO // 2) for oc in range(2)]
            fillmap = {}
            for pr in (1, 2):
                fills = kq_fills(pr + 1)
                fillmap[(pr, 0, 0)] = (fills[:2], 6)
                fillmap[(pr, 0, 1)] = (fills[2:4], 6)
                fillmap[(pr, 1, 0)] = (fills[4:6], 6)
                fillmap[(pr, 1, 1)] = (fills[6:], 6)
            fillmap[(3, 1, 0)] = (cfills[:10], 4)
            fillmap[(3, 1, 1)] = (cfills[10:], 4)
            for i, (pr, qc, hi) in enumerate(seq2):
                fills, fill_at = fillmap.get((pr, qc, hi), ((), 4))
                prep = (lambda j=i: qtp_prep(*seq2[j + 1])) \
                    if i + 1 < len(seq2) else None
                attn_iter(pr, qc, hi, fills=fills, fill_at=fill_at,
                          prep=prep)
            norm_part2()
            for so in range(NSO // 2, NSO):
                for oc in range(2):
                    c_fill(wo_sb, so, oc)

    nc.compile()
    return nc


def _get_nc():
    if "nc" not in _cache:
        _cache["nc"] = _build()
    return _cache["nc"]


def make_in_maps(q, k, v, mask, Wq, bq, Wk, bk, Wv, bv, Wo, bo):
    import ml_dtypes
    f = np.float32
    bf = ml_dtypes.bfloat16
    q = np.asarray(q, dtype=f)
    k = np.asarray(k, dtype=f)
    v = np.asarray(v, dtype=f)
    Wq = np.asarray(Wq, dtype=f)
    Wk = np.asarray(Wk, dtype=f)
    Wv = np.asarray(Wv, dtype=f)
    Wo = np.asarray(Wo, dtype=f)
    bq = np.asarray(bq, dtype=f)
    bk = np.asarray(bk, dtype=f)
    mask = np.asarray(mask)

    sel = np.zeros((P, 128), dtype=f)
    sel[0, 0:64] = 1.0
    sel[32, 64:128] = 1.0

    def chunk_w(Wc):
        r = Wc.reshape(NDM, P, NPC, P)
        return np.ascontiguousarray(r.transpose(2, 1, 0, 3))

    in_maps = []
    for c in range(NCORES):
        b, hg = divmod(c, 2)
        cols = slice(hg * PROJ, (hg + 1) * PROJ)
        mbias = np.where(mask[b, 0, 0, :] == 0, f(MASK_NEG), f(0.0)).astype(f)
        in_maps.append({
            "qT": np.ascontiguousarray(q[b].T).astype(bf),
            "kT": np.ascontiguousarray(k[b].T).astype(bf),
            "vT": np.ascontiguousarray(v[b].T).astype(bf),
            "wqr": chunk_w(Wq[:, cols]).astype(bf),
            "wkr": chunk_w(Wk[:, cols]).astype(bf),
            "wv": np.ascontiguousarray(Wv[:, cols]).astype(bf),
            "wo": np.ascontiguousarray(Wo[cols, :]).astype(bf),
            "bq2": np.ascontiguousarray(bq[cols].reshape(NPC, P).T),
            "bk2": np.ascontiguousarray(bk[cols].reshape(NPC, P).T),
            "mb": np.ascontiguousarray(mbias.reshape(NKC, P).T),
            "sel": sel.astype(bf),
            "vones": np.ones((P, NSO, HPC), dtype=bf),
        })
    return in_maps


def combine_outputs(parts, bv_Wo_bo):
    bv, Wo, bo = bv_Wo_bo
    bo_eff = (np.asarray(bv, np.float32) @ np.asarray(Wo, np.float32)
              + np.asarray(bo, np.float32))
    out = np.empty((B, S, D), dtype=np.float32)
    for b in range(B):
        out[b] = (parts[2 * b].astype(np.float32)
                  + parts[2 * b + 1].astype(np.float32) + bo_eff)
    return out


def _install_axon_ntff_hook():
    import sys
    import types
    if "antenv.axon_hooks" in sys.modules:
        return
    try:
        from trn_agent_boot.trn_boot import _ntff_profile_via_ctypes
        hook = _ntff_profile_via_ctypes("/opt/axon/libaxon_pjrt.so")
    except Exception:
        hook = None
    mod = types.ModuleType("antenv.axon_hooks")
    mod._hook = hook
    mod.get_axon_ntff_profile_hook = lambda: mod._hook
    mod.set_axon_ntff_profile_hook = lambda h: setattr(mod, "_hook", h)
    sys.modules["antenv.axon_hooks"] = mod
    import concourse.bass_utils as bu
    bu.upload_artifacts = lambda tmpdir: str(tmpdir)


def kernel(q, k, v, mask, Wq, bq, Wk, bk, Wv, bv, Wo, bo):
    from concourse.bass_utils import run_bass_kernel_spmd

    nc = _get_nc()
    in_maps = make_in_maps(q, k, v, mask, Wq, bq, Wk, bk, Wv, bv, Wo, bo)
    trace = bool(int(os.environ.get("KERNEL_TRACE", "0")))
    if trace:
        try:
            _install_axon_ntff_hook()
        except Exception:
            trace = False
    try:
        res = run_bass_kernel_spmd(
            nc, in_maps, list(range(NCORES)), trace=trace,
            tmpdir=os.environ.get("KERNEL_TRACE_DIR") or None)
    except Exception:
        if not trace:
            raise
        res = run_bass_kernel_spmd(nc, in_maps, list(range(NCORES)),
                                   trace=False)
    _cache["last_result"] = res
    parts = [res.results[c]["out"] for c in range(NCORES)]
    return combine_outputs(parts, (bv, Wo, bo))


# revision 3
# speedup vs baseline: 1.0212x; 1.0167x over previous
"""Multi-head attention (B=4, S=2048, D=1024, H=16) on 8 Trainium2 cores, v3.

Core c: batch c//2, head-group c%2 (8 heads, 512 proj dims).  v3 fuses all
phases into one dense PE/ACT stream built from single-head attention
iterations (pr, qc, hi):

  - scores: lhsT = packed K^T (both heads' dims, K=128 standard matmul),
    rhs = per-head zero-padded Q^T scratch (QTp; the two scratch buffers
    alternate by head parity so their zero halves persist).  No PE tiling
    modes anywhere (they hold the PE clock gate at 1.2 GHz).
  - AV + softmax denominator via the interleaved ones-column (M=65).
  - PSUM: psS 2x[128,1024] (scores/exp, double-buffered) + psAcc [65,1024]
    (AV accumulator) + psA 2x[128,512] dedicated to filler matmuls = 8 banks.
  - projections (K/Q for next pair, V JIT in pair 0, first half of the
    output projection in pair 3) run as atomic filler chunks in dedicated
    PSUM, so the exp stream's double-buffering is never disturbed.
  - copy-first normalization: l row + unnormalized A^T leave PSUM right
    after the last AV; selector-broadcast + reciprocal + scale run deferred
    inside the next iteration.  Output is written bf16 (host upcasts).
"""

import os
import numpy as np

B, S, D = 4, 2048, 1024
H, DK = 16, 64
P = 128
NCORES = 8
HPC = H // 2
PROJ = HPC * DK
NDM = D // P
NPC = PROJ // P
NSC = S // 512
NSO = S // P
NKC = S // P
VW = DK + 1             # 65 cols per head in V_sb: 64 data + ones

MASK_NEG = -30000.0

_cache = {}


def _build():
    import concourse.bacc as bacc
    import concourse.mybir as mybir
    import concourse.tile as tile
    from contextlib import ExitStack

    f32 = mybir.dt.float32
    bf16 = mybir.dt.bfloat16
    AF = mybir.ActivationFunctionType
    MUL = mybir.AluOpType.mult

    nc = bacc.Bacc("TRN2", target_bir_lowering=False, debug=False,
                   num_devices=NCORES)

    qT = nc.dram_tensor("qT", [D, S], bf16, kind="ExternalInput").ap()
    kT = nc.dram_tensor("kT", [D, S], bf16, kind="ExternalInput").ap()
    vT = nc.dram_tensor("vT", [D, S], bf16, kind="ExternalInput").ap()
    wqr = nc.dram_tensor("wqr", [NPC, P, NDM, P], bf16, kind="ExternalInput").ap()
    wkr = nc.dram_tensor("wkr", [NPC, P, NDM, P], bf16, kind="ExternalInput").ap()
    wv = nc.dram_tensor("wv", [D, PROJ], bf16, kind="ExternalInput").ap()
    wo = nc.dram_tensor("wo", [PROJ, D], bf16, kind="ExternalInput").ap()
    bq2 = nc.dram_tensor("bq2", [P, NPC], f32, kind="ExternalInput").ap()
    bk2 = nc.dram_tensor("bk2", [P, NPC], f32, kind="ExternalInput").ap()
    mb = nc.dram_tensor("mb", [P, NKC], f32, kind="ExternalInput").ap()
    sel = nc.dram_tensor("sel", [P, 128], bf16, kind="ExternalInput").ap()
    vones = nc.dram_tensor("vones", [P, NSO, HPC], bf16,
                           kind="ExternalInput").ap()
    out = nc.dram_tensor("out", [S, D], bf16, kind="ExternalOutput").ap()

    with tile.TileContext(nc) as tc, ExitStack() as ctx:
        cpool = ctx.enter_context(tc.tile_pool(name="const", bufs=1))
        sel_sb = cpool.tile([P, 128], bf16)
        nc.sync.dma_start(sel_sb[:], sel)
        mb_sb = cpool.tile([P, NKC], f32)
        nc.sync.dma_start(mb_sb[:], mb)
        bq_sb = cpool.tile([P, NPC], f32)
        nc.sync.dma_start(bq_sb[:], bq2)
        bk_sb = cpool.tile([P, NPC], f32)
        nc.sync.dma_start(bk_sb[:], bk2)
        Lsb = cpool.tile([P, 1024], bf16)
        nc.gpsimd.memset(Lsb[:], 0.0)
        # PE warm-up: ~30 matmuls on zeros keep the HAM activity window busy
        # during the input DMA wait, so the projection prologue runs at 2.4
        # GHz instead of the cold 1.2 GHz default.  Scoped pool: the bank is
        # returned before the attention PSUM pools are opened.
        with tc.tile_pool(name="warm", bufs=1, space="PSUM") as warm:
            wps = warm.tile([P, 512], f32)
            for _ in range(30):
                nc.tensor.matmul(wps, lhsT=Lsb[:, 0:128], rhs=Lsb[:, 0:512],
                                 start=True, stop=True)
        QTp = [cpool.tile([P, 1024], bf16, name=f"qtp{i}") for i in range(2)]
        nc.gpsimd.memset(QTp[0][:], 0.0)
        nc.gpsimd.memset(QTp[1][:], 0.0)

        rpool = ctx.enter_context(tc.tile_pool(name="res", bufs=1))
        kT_sb = rpool.tile([P, NDM, S], bf16)
        nc.sync.dma_start(kT_sb[:], kT.rearrange("(o p) s -> p o s", p=P))
        qT_sb = rpool.tile([P, NDM, S], bf16)
        nc.sync.dma_start(qT_sb[:], qT.rearrange("(o p) s -> p o s", p=P))
        QT_sb = rpool.tile([P, NPC, S], bf16)
        KT_sb = rpool.tile([P, NPC, S], bf16)
        AT_sb = rpool.tile([P, NPC, S], bf16)
        V_sb = rpool.tile([P, NSO, HPC * VW], bf16)
        nc.sync.dma_start(
            V_sb.rearrange("p n (h w) -> p n h w", w=VW)[:, :, :, DK], vones)

        wpool = ctx.enter_context(tc.tile_pool(name="w", bufs=2))
        stpool = ctx.enter_context(tc.tile_pool(name="st", bufs=2))
        epool = ctx.enter_context(tc.tile_pool(name="expS", bufs=2))
        rcpool = ctx.enter_context(tc.tile_pool(name="rc", bufs=1))
        bspool = ctx.enter_context(tc.tile_pool(name="bs", bufs=1))
        opool = ctx.enter_context(tc.tile_pool(name="ostage", bufs=2))
        psS = ctx.enter_context(tc.tile_pool(name="psS", bufs=2, space="PSUM"))
        psA = ctx.enter_context(tc.tile_pool(name="psA", bufs=2, space="PSUM"))
        psAcc = ctx.enter_context(tc.tile_pool(name="psAcc", bufs=1,
                                               space="PSUM"))

        def kq_fills(pr):
            wk_sb = wpool.tile([P, NDM, P], bf16, tag="w", name="wk_sb")
            nc.sync.dma_start(wk_sb[:], wkr[pr])
            wq_sb = wpool.tile([P, NDM, P], bf16, tag="w", name="wq_sb")
            nc.sync.dma_start(wq_sb[:], wqr[pr])

            def chunk(is_k, sc):
                w_sb = wk_sb if is_k else wq_sb
                src = kT_sb if is_k else qT_sb
                ps = psA.tile([P, 512], f32, tag="a", name="psp")
                for dc in range(NDM):
                    nc.tensor.matmul(
                        ps,
                        lhsT=w_sb[:, dc, :],
                        rhs=src[:, dc, sc * 512:(sc + 1) * 512],
                        start=(dc == 0), stop=(dc == NDM - 1),
                    )
                dst = KT_sb if is_k else QT_sb
                bias = bk_sb if is_k else bq_sb
                nc.vector.tensor_scalar_add(
                    dst[:, pr, sc * 512:(sc + 1) * 512], ps,
                    bias[:, pr:pr + 1])

            return [(lambda is_k=is_k, sc=sc: chunk(is_k, sc))
                    for is_k in (True, False) for sc in range(NSC)]

        def v_chunk(vT_sb, wv_sb, so):
            ps = psA.tile([P, 512], f32, tag="a", name="psv")
            for dc in range(NDM):
                nc.tensor.matmul(
                    ps,
                    lhsT=vT_sb[:, dc, so * P:(so + 1) * P],
                    rhs=wv_sb[:, dc, :],
                    start=(dc == 0), stop=(dc == NDM - 1),
                )
            nc.vector.tensor_copy(
                V_sb[:, so, :].rearrange("p (h w) -> p h w", w=VW)[:, :, 0:DK],
                ps.rearrange("p (h w) -> p h w", w=DK))

        def c_fill(wo_sb, so, oc):
            ps = psA.tile([P, 512], f32, tag="a", name="psc")
            for pc in range(NPC):
                nc.tensor.matmul(
                    ps,
                    lhsT=AT_sb[:, pc, so * P:(so + 1) * P],
                    rhs=wo_sb[:, pc, oc * 512:(oc + 1) * 512],
                    start=(pc == 0), stop=(pc == NPC - 1),
                )
            ost = opool.tile([P, 512], bf16, tag="o", name="ost")
            nc.vector.tensor_copy(ost, ps)
            nc.sync.dma_start(
                out[so * P:(so + 1) * P, oc * 512:(oc + 1) * 512], ost)

        pending = []

        def norm_part2():
            if not pending:
                return
            pr, qc = pending.pop()
            bc = psS.tile([P, 1024], f32, tag="s", name="bc")
            for sub in range(2):
                nc.tensor.matmul(
                    bc[:, sub * 512:(sub + 1) * 512],
                    lhsT=sel_sb[:],
                    rhs=Lsb[:, sub * 512:(sub + 1) * 512],
                    start=True, stop=True,
                )
            bs = bspool.tile([P, 1024], f32, tag="bs", name="bs")
            nc.vector.tensor_copy(bs, bc)
            rc = rcpool.tile([P, 1024], bf16, tag="rc", name="rc")
            with nc.allow_low_precision(reason="1/l in bf16 is within rtol"):
                nc.vector.reciprocal(rc, bs)
            for hi in range(2):
                lo = hi * 64
                dst = AT_sb[lo:lo + 64, pr, qc * 1024:(qc + 1) * 1024]
                nc.vector.tensor_tensor(dst, dst, rc[lo:lo + 64, :], MUL)

        def qtp_prep(pr, qc, hi):
            """Stage the zero-padded per-head Q^T slice for (pr, qc, hi)."""
            qoff = qc * 1024
            nc.vector.tensor_copy(QTp[hi][hi * 64:hi * 64 + 64, :],
                                  QT_sb[hi * 64:hi * 64 + 64, pr,
                                        qoff:qoff + 1024])

        def attn_iter(pr, qc, hi, pre=None, fills=(), fill_at=4, prep=None):
            fills = list(fills)
            h = 2 * pr + hi
            qoff = qc * 1024
            qtp = QTp[hi]
            avs = psAcc.tile([P, 1024], f32, tag="av", name="avs")
            for kc in range(NKC):
                sp = psS.tile([P, 1024], f32, tag="s", name="sp")
                for sub in range(2):
                    nc.tensor.matmul(
                        sp[:, sub * 512:(sub + 1) * 512],
                        lhsT=KT_sb[:, pr, kc * P:(kc + 1) * P],
                        rhs=qtp[:, sub * 512:(sub + 1) * 512],
                        start=True, stop=True,
                    )
                e = epool.tile([P, 1024], bf16, tag="e", name="e")
                nc.scalar.activation(
                    e, sp, AF.Exp,
                    bias=mb_sb[:, kc:kc + 1],
                    scale=float(1.0 / np.sqrt(DK)),
                )
                for sub in range(2):
                    nc.tensor.matmul(
                        avs[0:DK + 1, sub * 512:(sub + 1) * 512],
                        lhsT=V_sb[:, kc, h * VW:(h + 1) * VW],
                        rhs=e[:, sub * 512:(sub + 1) * 512],
                        start=(kc == 0), stop=(kc == NKC - 1),
                    )
                if pre is not None:
                    pre(kc)
                if kc == 3:
                    norm_part2()
                if kc == 8 and prep is not None:
                    prep()
                if kc >= fill_at and fills:
                    fills.pop(0)()
            while fills:
                fills.pop(0)()
            # evacuate PSUM in one staged copy; fan out off the critical path
            st = stpool.tile([P, 1024], bf16, tag="st", name="st")
            nc.vector.tensor_copy(st[0:DK + 1, :], avs[0:DK + 1, :])
            nc.vector.tensor_copy(Lsb[32 * hi:32 * hi + 1, :],
                                  st[DK:DK + 1, :])
            nc.vector.tensor_copy(
                AT_sb[64 * hi:64 * hi + 64, pr, qoff:qoff + 1024],
                st[0:DK, :])
            if hi == 1:
                pending.append((pr, qc))

        # ---------------- schedule ----------------
        fills0 = kq_fills(0)
        with tc.tile_pool(name="vres", bufs=1) as vpool:
            vT_sb = vpool.tile([P, NDM, S], bf16)
            nc.sync.dma_start(vT_sb[:], vT.rearrange("(o p) s -> p o s", p=P))
            wv_sb = vpool.tile([P, NDM, PROJ], bf16)
            nc.sync.dma_start(wv_sb[:], wv.rearrange("(o p) n -> p o n", p=P))

            for f in fills0:
                f()
            v_chunk(vT_sb, wv_sb, 0)

            def pre_v(kc):
                if kc + 1 < NSO:
                    v_chunk(vT_sb, wv_sb, kc + 1)

            fills1 = kq_fills(1)
            seq = [(0, 0, 0), (0, 0, 1), (0, 1, 0), (0, 1, 1)]
            preps = [
                (lambda i=i: qtp_prep(*seq[i + 1])) for i in range(3)
            ] + [lambda: qtp_prep(1, 0, 0)]
            qtp_prep(0, 0, 0)
            attn_iter(0, 0, 0, pre=pre_v, prep=preps[0])
            attn_iter(0, 0, 1, fills=fills1[:3], fill_at=5, prep=preps[1])
            attn_iter(0, 1, 0, fills=fills1[3:6], fill_at=5, prep=preps[2])
            attn_iter(0, 1, 1, fills=fills1[6:], fill_at=5, prep=preps[3])

        with tc.tile_pool(name="wores", bufs=1) as wopool:
            wo_sb = wopool.tile([P, NPC, D], bf16)
            nc.sync.dma_start(wo_sb[:], wo.rearrange("(o p) n -> p o n", p=P))

            seq2 = [(pr, qc, hi) for pr in (1, 2, 3)
                    for qc in (0, 1) for hi in (0, 1)]
            cfills = [(lambda so=so, oc=oc: c_fill(wo_sb, so, oc))
                      for so in range(NSO // 2) for oc in range(2)]
            fillmap = {}
            for pr in (1, 2):
                fills = kq_fills(pr + 1)
                fillmap[(pr, 0, 0)] = (fills[:2], 6)
                fillmap[(pr, 0, 1)] = (fills[2:4], 6)
                fillmap[(pr, 1, 0)] = (fills[4:6], 6)
                fillmap[(pr, 1, 1)] = (fills[6:], 6)
            # C fills wait on the (3,0) norm chain (~9us after its kc==3
            # trigger), so start them late in (3,1,0).
            fillmap[(3, 1, 0)] = (cfills[:4], 12)
            fillmap[(3, 1, 1)] = (cfills[4:], 4)
            for i, (pr, qc, hi) in enumerate(seq2):
                fills, fill_at = fillmap.get((pr, qc, hi), ((), 4))
                prep = (lambda j=i: qtp_prep(*seq2[j + 1])) \
                    if i + 1 < len(seq2) else None
                attn_iter(pr, qc, hi, fills=fills, fill_at=fill_at,
                          prep=prep)
            norm_part2()
            for so in range(NSO // 2, NSO):
                for oc in range(2):
                    c_fill(wo_sb, so, oc)

    nc.compile()
    return nc


def _get_nc():
    if "nc" not in _cache:
        _cache["nc"] = _build()
    return _cache["nc"]


def make_in_maps(q, k, v, mask, Wq, bq, Wk, bk, Wv, bv, Wo, bo):
    import ml_dtypes
    f = np.float32
    bf = ml_dtypes.bfloat16
    q = np.asarray(q, dtype=f)
    k = np.asarray(k, dtype=f)
    v = np.asarray(v, dtype=f)
    Wq = np.asarray(Wq, dtype=f)
    Wk = np.asarray(Wk, dtype=f)
    Wv = np.asarray(Wv, dtype=f)
    Wo = np.asarray(Wo, dtype=f)
    bq = np.asarray(bq, dtype=f)
    bk = np.asarray(bk, dtype=f)
    mask = np.asarray(mask)

    sel = np.zeros((P, 128), dtype=f)
    sel[0, 0:64] = 1.0
    sel[32, 64:128] = 1.0

    def chunk_w(Wc):
        r = Wc.reshape(NDM, P, NPC, P)
        return np.ascontiguousarray(r.transpose(2, 1, 0, 3))

    in_maps = []
    for c in range(NCORES):
        b, hg = divmod(c, 2)
        cols = slice(hg * PROJ, (hg + 1) * PROJ)
        mbias = np.where(mask[b, 0, 0, :] == 0, f(MASK_NEG), f(0.0)).astype(f)
        in_maps.append({
            "qT": np.ascontiguousarray(q[b].T).astype(bf),
            "kT": np.ascontiguousarray(k[b].T).astype(bf),
            "vT": np.ascontiguousarray(v[b].T).astype(bf),
            "wqr": chunk_w(Wq[:, cols]).astype(bf),
            "wkr": chunk_w(Wk[:, cols]).astype(bf),
            "wv": np.ascontiguousarray(Wv[:, cols]).astype(bf),
            "wo": np.ascontiguousarray(Wo[cols, :]).astype(bf),
            "bq2": np.ascontiguousarray(bq[cols].reshape(NPC, P).T),
            "bk2": np.ascontiguousarray(bk[cols].reshape(NPC, P).T),
            "mb": np.ascontiguousarray(mbias.reshape(NKC, P).T),
            "sel": sel.astype(bf),
            "vones": np.ones((P, NSO, HPC), dtype=bf),
        })
    return in_maps


def combine_outputs(parts, bv_Wo_bo):
    bv, Wo, bo = bv_Wo_bo
    bo_eff = (np.asarray(bv, np.float32) @ np.asarray(Wo, np.float32)
              + np.asarray(bo, np.float32))
    out = np.empty((B, S, D), dtype=np.float32)
    for b in range(B):
        out[b] = (parts[2 * b].astype(np.float32)
                  + parts[2 * b + 1].astype(np.float32) + bo_eff)
    return out


def _install_axon_ntff_hook():
    import sys
    import types
    if "antenv.axon_hooks" in sys.modules:
        return
    try:
        from trn_agent_boot.trn_boot import _ntff_profile_via_ctypes
        hook = _ntff_profile_via_ctypes("/opt/axon/libaxon_pjrt.so")
    except Exception:
        hook = None
    mod = types.ModuleType("antenv.axon_hooks")
    mod._hook = hook
    mod.get_axon_ntff_profile_hook = lambda: mod._hook
    mod.set_axon_ntff_profile_hook = lambda h: setattr(mod, "_hook", h)
    sys.modules["antenv.axon_hooks"] = mod
    import concourse.bass_utils as bu
    bu.upload_artifacts = lambda tmpdir: str(tmpdir)


def kernel(q, k, v, mask, Wq, bq, Wk, bk, Wv, bv, Wo, bo):
    from concourse.bass_utils import run_bass_kernel_spmd

    nc = _get_nc()
    in_maps = make_in_maps(q, k, v, mask, Wq, bq, Wk, bk, Wv, bv, Wo, bo)
    trace = bool(int(os.environ.get("KERNEL_TRACE", "0")))
    if trace:
        try:
            _install_axon_ntff_hook()
        except Exception:
            trace = False
    try:
        res = run_bass_kernel_spmd(
            nc, in_maps, list(range(NCORES)), trace=trace,
            tmpdir=os.environ.get("KERNEL_TRACE_DIR") or None)
    except Exception:
        if not trace:
            raise
        res = run_bass_kernel_spmd(nc, in_maps, list(range(NCORES)),
                                   trace=False)
    _cache["last_result"] = res
    parts = [res.results[c]["out"] for c in range(NCORES)]
    return combine_outputs(parts, (bv, Wo, bo))
